# revision 1
# baseline (speedup 1.0000x reference)
"""Trainium2 Bass kernel for a 2-layer GAT cost model (gnn_message_passing).

Strategy (8 NeuronCores, SPMD):
  - Nodes partitioned by range: core p owns [p*12500, (p+1)*12500), padded to
    12544 = 98 blocks x 128.
  - Edges (incl. self-loops) partitioned by destination core, grouped by
    128-node destination block, padded per block to T_BLK tiles of 128 edges.
  - Per conv: each core builds table rows xg[n] = [xp(128)|a_src(4)|a_dst(4)]
    for its own nodes via one matmul with host-prefolded weights
    [W | W@att_src^T | W@att_dst^T], then the table is AllGathered.
  - Aggregation per destination block: batched indirect-DMA gather of source
    rows, attention ealpha = exp(leakyrelu(a_src+a_dst)) (softmax without the
    per-segment max shift - mathematically identical, fp-wise near-identical),
    one-hot selection matrix S[e,m] = (dst_local(e)==m) built by is_equal
    against an iota row, messages scaled by ealpha, and a single PSUM
    accumulating matmul per 128-edge tile computing both the weighted message
    sum and the softmax denominators (ealpha written into rhs cols 128:132).
  - Epilogue: divide by denominator, +bias, ReLU; PE-transpose to keep a
    feature-major copy of activations for the next layer's matmuls.
  - Sum-pool via one-hot-of-batch matmuls + AllReduce; tiny head MLP
    replicated on every core.
"""
import functools
from contextlib import ExitStack

import numpy as np

import concourse.bass as bass
import concourse.tile as tile
from concourse import bacc, mybir
from concourse.bass_utils import run_bass_kernel_spmd

# ---- problem constants (hardcoded; kernel.py must be self-contained) ----
N = 100000
E = 1600000
B = 64
NODE_DIM, CFG_DIM, HW_DIM, G_DIM = 32, 16, 8, 16
HID, HEADS = 128, 4
HD = HID // HEADS
IN_DIM = NODE_DIM + CFG_DIM + HW_DIM          # 56
NC = 8
NLOC = N // NC                                 # 12500
NB = 98                                        # dst blocks per core
NPAD = NB * 128                                # 12544
F = HID + 2 * HEADS                            # 136 table row width
P = 128
f32 = mybir.dt.float32
i32 = mybir.dt.int32


# --------------------------------------------------------------------------
# host-side prep
# --------------------------------------------------------------------------
def _prep_graph(edge_index):
    src = np.concatenate([np.asarray(edge_index[0]), np.arange(N)]).astype(np.int64)
    dst = np.concatenate([np.asarray(edge_index[1]), np.arange(N)]).astype(np.int64)
    core = dst // NLOC
    per_core = []
    tmax = 0
    for p in range(NC):
        sel = core == p
        d_loc = (dst[sel] - p * NLOC).astype(np.int32)
        s_glob = src[sel].astype(np.int64)
        blk = d_loc >> 7
        counts = np.bincount(blk, minlength=NB)
        tmax = max(tmax, int(np.ceil(counts.max() / 128)))
        per_core.append((d_loc, s_glob, blk, counts))
    T = tmax
    epb = T * 128
    outs = []
    for p in range(NC):
        d_loc, s_glob, blk, counts = per_core[p]
        order = np.argsort(blk, kind="stable")
        c_arr = np.full(NB * epb, -1.0, np.float32)
        g_arr = np.zeros(NB * epb, np.int32)
        d_arr = np.zeros(NB * epb, np.int32)
        starts = np.zeros(NB + 1, np.int64)
        np.cumsum(counts, out=starts[1:])
        # position of each (sorted) edge inside its block
        pos_in_blk = np.arange(len(order)) - starts[blk[order]]
        slot = blk[order] * epb + pos_in_blk
        c_arr[slot] = (d_loc[order] & 127).astype(np.float32)
        gidx = (s_glob // NLOC) * NPAD + (s_glob % NLOC)
        g_arr[slot] = gidx[order].astype(np.int32)
        d_arr[slot] = (p * NPAD + d_loc[order]).astype(np.int32)
        # device layout: element (partition, b*T + t) = edge slot b*epb + t*128 + partition
        c_dev = c_arr.reshape(NB, T, 128).transpose(2, 0, 1).reshape(128, NB * T)
        g_dev = g_arr.reshape(NB, T, 128).transpose(2, 0, 1).reshape(128, NB * T)
        d_dev = d_arr.reshape(NB, T, 128).transpose(2, 0, 1).reshape(128, NB * T)
        outs.append((np.ascontiguousarray(c_dev), np.ascontiguousarray(g_dev),
                     np.ascontiguousarray(d_dev)))
    return T, outs


def _prep_inputs(inputs):
    x = np.asarray(inputs["x"], np.float32)
    s2 = np.asarray(inputs["s"], np.float32).reshape(B, CFG_DIM)
    h2 = np.asarray(inputs["h"], np.float32).reshape(B, HW_DIM)
    g2 = np.asarray(inputs["g"], np.float32).reshape(B, G_DIM)
    batch = np.asarray(inputs["batch"], np.int64)

    T, graph = _prep_graph(inputs["edge_index"])

    def wcat(W, as_, ad_):
        W = np.asarray(W, np.float32)
        wa_s = np.stack([W[:, h * HD:(h + 1) * HD] @ np.asarray(as_, np.float32)[h]
                         for h in range(HEADS)], axis=1)
        wa_d = np.stack([W[:, h * HD:(h + 1) * HD] @ np.asarray(ad_, np.float32)[h]
                         for h in range(HEADS)], axis=1)
        return np.ascontiguousarray(np.concatenate([W, wa_s, wa_d], axis=1))

    com = {
        "iota128": np.tile(np.arange(P, dtype=np.float32), (P, 1)),
        "iota64": np.tile(np.arange(B, dtype=np.float32), (P, 1)),
        "ident": np.eye(P, dtype=np.float32),
        "Win": np.asarray(inputs["Win"], np.float32),
        "binc": np.asarray(inputs["bin_"], np.float32).reshape(P, 1),
        "Wc0": wcat(inputs["W0"], inputs["as0"], inputs["ad0"]),
        "Wc1": wcat(inputs["W1"], inputs["as1"], inputs["ad1"]),
        "b0r": np.tile(np.asarray(inputs["b0"], np.float32), (P, 1)),
        "b1r": np.tile(np.asarray(inputs["b1"], np.float32), (P, 1)),
        "Wh1a": np.asarray(inputs["Wh1"], np.float32)[:HID],
        "Wh1b": np.ascontiguousarray(np.asarray(inputs["Wh1"], np.float32)[HID:]),
        "bh1r": np.tile(np.asarray(inputs["bh1"], np.float32), (B, 1)),
        "Wh2a": np.asarray(inputs["Wh2"], np.float32)[:128],
        "Wh2b": np.ascontiguousarray(np.asarray(inputs["Wh2"], np.float32)[128:]),
        "bh2r": np.tile(np.asarray(inputs["bh2"], np.float32), (B, 1)),
        "Wh3": np.asarray(inputs["Wh3"], np.float32),
        "bh3r": np.tile(np.asarray(inputs["bh3"], np.float32), (B, 1)),
        "sgT": np.ascontiguousarray(
            np.concatenate([s2, h2, g2], axis=1).T),
    }
    com = {k: np.ascontiguousarray(v, np.float32) for k, v in com.items()}

    sh = np.concatenate([s2[batch], h2[batch]], axis=1).astype(np.float32)  # [N,24]
    in_maps = []
    for p in range(NC):
        lo, hi = p * NLOC, (p + 1) * NLOC
        xT = np.zeros((NODE_DIM, NPAD), np.float32)
        xT[:, :NLOC] = x[lo:hi].T
        shT = np.zeros((CFG_DIM + HW_DIM, NPAD), np.float32)
        shT[:, :NLOC] = sh[lo:hi].T
        bat = np.full(NPAD, 999.0, np.float32)
        bat[:NLOC] = batch[lo:hi].astype(np.float32)
        c_dev, g_dev, d_dev = graph[p]
        m = dict(com)
        m["xT"] = xT
        m["shT"] = shT
        m["bat"] = np.ascontiguousarray(bat.reshape(NB, 128).T)
        m["cL"] = c_dev
        m["gi"] = g_dev
        m["gd"] = d_dev
        in_maps.append(m)
    return T, in_maps


# --------------------------------------------------------------------------
# device program
# --------------------------------------------------------------------------
@functools.lru_cache(maxsize=8)
def _build(T, stages=99, nbdbg=NB, sub=9):
    nc = bacc.Bacc("TRN2", target_bir_lowering=False, debug=False)
    ET = NB * T  # edge tiles per core

    def din(name, shape, dt=f32):
        return nc.dram_tensor(name, list(shape), dt, kind="ExternalInput")

    xT_h = din("xT", (NODE_DIM, NPAD))
    shT_h = din("shT", (CFG_DIM + HW_DIM, NPAD))
    bat_h = din("bat", (P, NB))
    cL_h = din("cL", (P, ET))
    gi_h = din("gi", (P, ET), i32)
    gd_h = din("gd", (P, ET), i32)
    iota128_h = din("iota128", (P, P))
    iota64_h = din("iota64", (P, B))
    ident_h = din("ident", (P, P))
    Win_h = din("Win", (IN_DIM, HID))
    binc_h = din("binc", (P, 1))
    Wc_h = [din("Wc0", (HID, F)), din("Wc1", (HID, F))]
    br_h = [din("b0r", (P, HID)), din("b1r", (P, HID))]
    Wh1a_h = din("Wh1a", (HID, 256))
    Wh1b_h = din("Wh1b", (40, 256))
    bh1r_h = din("bh1r", (B, 256))
    Wh2a_h = din("Wh2a", (128, 128))
    Wh2b_h = din("Wh2b", (128, 128))
    bh2r_h = din("bh2r", (B, 128))
    Wh3_h = din("Wh3", (128, 1))
    bh3r_h = din("bh3r", (B, 1))
    sgT_h = din("sgT", (40, B))

    out_h = nc.dram_tensor("out", [B, 1], f32, kind="ExternalOutput")

    xg_loc = [nc.dram_tensor(f"xg{k}_loc", [NPAD, F], f32) for k in range(2)]
    xg_full = [nc.dram_tensor(f"xg{k}_full", [NC * NPAD, F], f32,
                              addr_space="Shared") for k in range(2)]
    pool_loc = nc.dram_tensor("pool_loc", [B, HID], f32)
    dbg = nc.dram_tensor("dbg", [P, 16], f32)
    pool_full = nc.dram_tensor("pool_full", [B, HID], f32, addr_space="Shared")

    AG = "AllGather"
    AR = "AllReduce"
    rg = [list(range(NC))]

    with tile.TileContext(nc) as tc, ExitStack() as ctx:
        const = ctx.enter_context(tc.tile_pool(name="const", bufs=1))
        inp = ctx.enter_context(tc.tile_pool(name="inp", bufs=3))
        tblp = ctx.enter_context(tc.tile_pool(name="tblp", bufs=4))
        gxp = ctx.enter_context(tc.tile_pool(name="gxp", bufs=3))
        sp = ctx.enter_context(tc.tile_pool(name="sp", bufs=3))
        ealp = ctx.enter_context(tc.tile_pool(name="ealp", bufs=3))
        nmp = ctx.enter_context(tc.tile_pool(name="nmp", bufs=3))
        smallp = ctx.enter_context(tc.tile_pool(name="smallp", bufs=4))
        psA = ctx.enter_context(tc.tile_pool(name="psA", bufs=2, space="PSUM"))
        psT = ctx.enter_context(tc.tile_pool(name="psT", bufs=2, space="PSUM"))
        psP = ctx.enter_context(tc.tile_pool(name="psP", bufs=1, space="PSUM"))

        def load_const(h, shape, dt=f32):
            t = const.tile(list(shape), dt, tag=h.name)
            nc.sync.dma_start(t[:], h[:])
            return t

        xhT = const.tile([P, NPAD], f32, tag="xhT")          # resident activations^T
        cL = load_const(cL_h, (P, ET))
        gi = load_const(gi_h, (P, ET), i32)
        gd = load_const(gd_h, (P, ET), i32)
        bat = load_const(bat_h, (P, NB))
        iota128 = load_const(iota128_h, (P, P))
        iota64 = load_const(iota64_h, (P, B))
        ident = load_const(ident_h, (P, P))
        WinA = const.tile([NODE_DIM, HID], f32, tag="WinA")
        nc.sync.dma_start(WinA[:], Win_h[0:NODE_DIM, :])
        WinB = const.tile([CFG_DIM + HW_DIM, HID], f32, tag="WinB")
        nc.sync.dma_start(WinB[:], Win_h[NODE_DIM:IN_DIM, :])
        binc = load_const(binc_h, (P, 1))
        Wc = [load_const(Wc_h[k], (HID, F)) for k in range(2)]
        br = [load_const(br_h[k], (P, HID)) for k in range(2)]
        Wh1a = load_const(Wh1a_h, (HID, 256))
        Wh1b = load_const(Wh1b_h, (40, 256))
        bh1r = load_const(bh1r_h, (B, 256))
        Wh2a = load_const(Wh2a_h, (128, 128))
        Wh2b = load_const(Wh2b_h, (128, 128))
        bh2r = load_const(bh2r_h, (B, 128))
        Wh3 = load_const(Wh3_h, (128, 1))
        bh3r = load_const(bh3r_h, (B, 1))
        sgT = load_const(sgT_h, (40, B))

        # ---------------- stage 0: input MLP (feature-major) ----------------
        CH = 512
        for c0 in range(0, NPAD, CH):
            w = min(CH, NPAD - c0)
            xc = inp.tile([NODE_DIM, CH], f32, tag="xc")
            nc.sync.dma_start(xc[:, :w], xT_h[:, c0:c0 + w])
            sc = inp.tile([CFG_DIM + HW_DIM, CH], f32, tag="sc")
            nc.sync.dma_start(sc[:, :w], shT_h[:, c0:c0 + w])
            pin = psT.tile([P, CH], f32, tag="ps1", space="PSUM")
            nc.tensor.matmul(pin[:, :w], lhsT=WinA[:], rhs=xc[:, :w],
                             start=True, stop=False)
            nc.tensor.matmul(pin[:, :w], lhsT=WinB[:], rhs=sc[:, :w],
                             start=False, stop=True)
            nc.scalar.activation(xhT[:, c0:c0 + w], pin[:, :w],
                                 mybir.ActivationFunctionType.Relu, bias=binc[:])

        pool_ps = psP.tile([B, HID], f32, space="PSUM")

        # ---------------- two GAT convs ----------------
        for k in range(2 if stages >= 4 else (1 if stages >= 1 else 0)):
            # A) local gather-table rows [xp | a_src | a_dst]
            for j in range(NB):
                pt = psT.tile([P, F], f32, tag="ps1", space="PSUM")
                nc.tensor.matmul(pt[:], lhsT=xhT[:, j * P:(j + 1) * P], rhs=Wc[k][:],
                                 start=True, stop=True)
                tb = tblp.tile([P, F], f32, tag="tb")
                nc.scalar.copy(tb[:], pt[:])
                nc.sync.dma_start(xg_loc[k][j * P:(j + 1) * P, :], tb[:])

            # B) AllGather the table
            if stages < 2 and k == 0:
                continue
            nc.gpsimd.collective_compute(
                AG, mybir.AluOpType.bypass, replica_groups=rg,
                ins=[xg_loc[k][:]], outs=[xg_full[k][:]],
            )

            # C) aggregate per destination block
            if stages < 3 and k == 0:
                continue
            for b in range(nbdbg):
                gx = gxp.tile([P, T, F], f32, tag="gx")
                for t in range(T):
                    nc.gpsimd.indirect_dma_start(
                        out=gx[:, t, :], out_offset=None, in_=xg_full[k][:],
                        in_offset=bass.IndirectOffsetOnAxis(
                            ap=gi[:, b * T + t:b * T + t + 1], axis=0),
                    )
                if sub < 2:
                    nc.sync.dma_start(dbg[:], gx[:, 0, 0:16])
                    continue
                # a_dst of the destination nodes (second, narrow gather)
                gx2 = gxp.tile([P, T, HEADS], f32, tag="gx2")
                for t in range(T):
                    nc.gpsimd.indirect_dma_start(
                        out=gx2[:, t, :], out_offset=None, in_=xg_full[k][:],
                        in_offset=bass.IndirectOffsetOnAxis(
                            ap=gd[:, b * T + t:b * T + t + 1], axis=0),
                        element_offset=HID + 4,
                    )
                if sub < 3:
                    nc.sync.dma_start(dbg[:], gx2[:, 0:4, :])
                    continue
                # attention coefs: asrc[src] + adst[dst]
                eal = ealp.tile([P, T * HEADS], f32, tag="eal")
                ealv = eal[:].rearrange("p (t h) -> p t h", t=T)
                nc.vector.tensor_tensor(out=ealv, in0=gx[:, :, HID:HID + 4],
                                        in1=gx2[:], op=mybir.AluOpType.add)
                t02 = ealp.tile([P, T * HEADS], f32, tag="t02")
                nc.vector.tensor_scalar(t02[:], eal[:], 0.2, None,
                                        mybir.AluOpType.mult)
                nc.vector.tensor_tensor(out=eal[:], in0=eal[:], in1=t02[:],
                                        op=mybir.AluOpType.max)
                # exp -> rhs cols 128:132 of gx
                nc.scalar.activation(gx[:, :, HID:HID + 4], eal[:].rearrange(
                    "p (t h) -> p t h", t=T), mybir.ActivationFunctionType.Exp)
                if sub < 4:
                    nc.sync.dma_start(dbg[:], gx[:, 0, HID:HID+16])
                    continue
                # selection matrix
                S = sp.tile([P, T * P], f32, tag="S")
                nc.vector.tensor_tensor(
                    out=S[:].rearrange("p (t m) -> p t m", t=T),
                    in0=iota128[:].unsqueeze(1).to_broadcast([P, T, P]),
                    in1=cL[:, b * T:(b + 1) * T].unsqueeze(2).to_broadcast([P, T, P]),
                    op=mybir.AluOpType.is_equal)
                if sub < 5:
                    nc.sync.dma_start(dbg[:], S[:, 0:16])
                    continue
                # scale messages in place
                nc.vector.tensor_tensor(
                    out=gx[:, :, 0:HID].rearrange("p t (h d) -> p t h d", h=HEADS),
                    in0=gx[:, :, 0:HID].rearrange("p t (h d) -> p t h d", h=HEADS),
                    in1=gx[:, :, HID:HID + 4].unsqueeze(3).to_broadcast(
                        [P, T, HEADS, HD]),
                    op=mybir.AluOpType.mult)
                if sub < 6:
                    nc.sync.dma_start(dbg[:], gx[:, 0, 0:16])
                    continue
                # accumulate
                pa = psA.tile([P, HID + 4], f32, tag="pa", space="PSUM")
                for t in range(T):
                    nc.tensor.matmul(pa[:], lhsT=S[:, t * P:(t + 1) * P],
                                     rhs=gx[:, t, 0:HID + 4],
                                     start=(t == 0), stop=(t == T - 1))
                if sub < 7:
                    dcp = nmp.tile([P, 16], f32, tag="dcp")
                    nc.vector.tensor_copy(dcp[:], pa[:, 0:16])
                    nc.sync.dma_start(dbg[:], dcp[:])
                    continue
                # epilogue: xh = relu(msg/denom + bias)
                den = smallp.tile([P, HEADS], f32, tag="den")
                nc.vector.tensor_scalar(den[:], pa[:, HID:HID + 4], 1e-16, None,
                                        mybir.AluOpType.add)
                denr = smallp.tile([P, HEADS], f32, tag="denr")
                nc.vector.reciprocal(denr[:], den[:])
                xn = nmp.tile([P, HID], f32, tag="xn")
                for h in range(HEADS):
                    nc.vector.tensor_scalar(xn[:, h * HD:(h + 1) * HD],
                                            pa[:, h * HD:(h + 1) * HD],
                                            denr[:, h:h + 1], None,
                                            mybir.AluOpType.mult)
                nc.vector.tensor_tensor(out=xn[:], in0=xn[:], in1=br[k][:],
                                        op=mybir.AluOpType.add)
                nc.scalar.activation(xn[:], xn[:],
                                     mybir.ActivationFunctionType.Relu)
                if sub < 8:
                    nc.sync.dma_start(dbg[:], xn[:, 0:16])
                    continue
                if k == 0:
                    ptr = psT.tile([P, P], f32, tag="ps1", space="PSUM")
                    nc.tensor.transpose(out=ptr[:], in_=xn[:], identity=ident[:])
                    nc.scalar.copy(xhT[:, b * P:(b + 1) * P], ptr[:])
                else:
                    Spool = smallp.tile([P, B], f32, tag="Spool")
                    nc.vector.tensor_scalar(Spool[:], iota64[:], bat[:, b:b + 1],
                                            None, mybir.AluOpType.is_equal)
                    nc.tensor.matmul(pool_ps[:], lhsT=Spool[:], rhs=xn[:],
                                     start=(b == 0), stop=(b == nbdbg - 1))

        # ---------------- pooling all-reduce + head MLP ----------------
        if stages < 5:
            zz = smallp.tile([B, 1], f32, tag="zz")
            nc.vector.memset(zz[:], 0.0)
            nc.sync.dma_start(out_h[:], zz[:])
        do_head = stages >= 5
        if do_head:
            psb = smallp.tile([B, HID], f32, tag="psb")
            nc.scalar.copy(psb[:], pool_ps[:])
            nc.sync.dma_start(pool_loc[:], psb[:])
            nc.gpsimd.collective_compute(
                AR, mybir.AluOpType.add, replica_groups=rg,
                ins=[pool_loc[:]], outs=[pool_full[:]],
            )
            pooled = smallp.tile([B, HID], f32, tag="pooled")
            nc.sync.dma_start(pooled[:], pool_full[:])

            ptp = psT.tile([P, B], f32, tag="ps1", space="PSUM")
            nc.tensor.transpose(out=ptp[:], in_=pooled[:], identity=ident[0:B, 0:B])
            pT = smallp.tile([P, B], f32, tag="pT")
            nc.scalar.copy(pT[:], ptp[:])

            z1p = psT.tile([B, 256], f32, tag="ps1", space="PSUM")
            nc.tensor.matmul(z1p[:], lhsT=pT[:], rhs=Wh1a[:], start=True, stop=False)
            nc.tensor.matmul(z1p[:], lhsT=sgT[:], rhs=Wh1b[:], start=False, stop=True)
            z1 = smallp.tile([B, 256], f32, tag="z1")
            nc.vector.tensor_tensor(out=z1[:], in0=z1p[:], in1=bh1r[:],
                                    op=mybir.AluOpType.add)
            nc.scalar.activation(z1[:], z1[:], mybir.ActivationFunctionType.Relu)

            z1Ta = psT.tile([P, B], f32, tag="ps1", space="PSUM")
            nc.tensor.transpose(out=z1Ta[:], in_=z1[:, 0:128], identity=ident[0:B, 0:B])
            z1Tb = psT.tile([P, B], f32, tag="ps1", space="PSUM")
            nc.tensor.transpose(out=z1Tb[:], in_=z1[:, 128:256], identity=ident[0:B, 0:B])
            z1Tas = smallp.tile([P, B], f32, tag="z1Tas")
            nc.scalar.copy(z1Tas[:], z1Ta[:])
            z1Tbs = smallp.tile([P, B], f32, tag="z1Tbs")
            nc.scalar.copy(z1Tbs[:], z1Tb[:])

            z2p = psT.tile([B, 128], f32, tag="ps1", space="PSUM")
            nc.tensor.matmul(z2p[:], lhsT=z1Tas[:], rhs=Wh2a[:], start=True, stop=False)
            nc.tensor.matmul(z2p[:], lhsT=z1Tbs[:], rhs=Wh2b[:], start=False, stop=True)
            z2 = smallp.tile([B, 128], f32, tag="z2")
            nc.vector.tensor_tensor(out=z2[:], in0=z2p[:], in1=bh2r[:],
                                    op=mybir.AluOpType.add)
            nc.scalar.activation(z2[:], z2[:], mybir.ActivationFunctionType.Relu)

            z2T = psT.tile([P, B], f32, tag="ps1", space="PSUM")
            nc.tensor.transpose(out=z2T[:], in_=z2[:], identity=ident[0:B, 0:B])
            z2Ts = smallp.tile([P, B], f32, tag="z2Ts")
            nc.scalar.copy(z2Ts[:], z2T[:])

            z3p = psT.tile([B, 1], f32, tag="ps1", space="PSUM")
            nc.tensor.matmul(z3p[:], lhsT=z2Ts[:], rhs=Wh3[:], start=True, stop=True)
            z3 = smallp.tile([B, 1], f32, tag="z3")
            nc.vector.tensor_tensor(out=z3[:], in0=z3p[:], in1=bh3r[:],
                                    op=mybir.AluOpType.add)
            nc.sync.dma_start(out_h[:], z3[:])

    nc.compile()
    return nc


# --------------------------------------------------------------------------
# entry point
# --------------------------------------------------------------------------
def kernel(**inputs) -> np.ndarray:
    T, in_maps = _prep_inputs(inputs)
    nc = _build(T)
    res = run_bass_kernel_spmd(nc, in_maps, core_ids=list(range(NC)))
    return np.asarray(res.results[0]["out"], np.float32).reshape(B)



# revision 4
# speedup vs baseline: 2.2270x; 2.2270x over previous
"""Trainium2 Bass kernel for a 2-layer GAT cost model (gnn_message_passing).

Strategy (8 NeuronCores, SPMD), v2:
  - Nodes partitioned by range: core p owns [p*12500, (p+1)*12500), padded to
    12544 = 98 blocks x 128.
  - Non-self-loop edges partitioned by destination core, grouped by 128-node
    destination block, padded per block to T tiles of 128 edges.  Self-loop
    edges never leave the core: their contribution is added in the epilogue
    from the resident local table (no gather, no matmul).
  - Per conv: each core builds table rows [xp(128)|asrc(4)] (bf16) for its
    own nodes via one matmul with host-prefolded weights [W | W@as^T | W@ad^T]
    (adst kept resident in SBUF, not shipped); table AllGathered in bf16.
  - Aggregation per destination block:
      * per-tile indirect-DMA gather of source rows (bf16, 264B rows) -- the
        only per-edge descriptor work on GpSimd;
      * per-edge adst WITHOUT a second gather: S^T[m,e] = (cL[e]==m) built by
        a K=1 PE broadcast of the host-shipped cL row + DVE is_equal, then
        adst_e = S_T^T @ adstL via tiny per-tile matmuls;
      * ealpha = exp(leakyrelu(asrc+adst)) (softmax without max shift),
        written into gather rows cols 128:132;
      * one-hot S[e,m] built by DVE is_equal (bf16); messages scaled by
        ealpha; single PSUM-accumulating matmul chain per block computes both
        weighted message sums and softmax denominators.
  - Epilogue: add self-loop terms, divide by denominator, +bias, ReLU;
    PE-transpose keeps a feature-major bf16 copy for the next layer.
  - Sum-pool via one-hot-of-batch matmuls + AllReduce; tiny fp32 head MLP
    replicated on every core.
"""
import functools
from contextlib import ExitStack

import numpy as np
import ml_dtypes

import concourse.bass as bass
import concourse.tile as tile
from concourse import bacc, mybir
from concourse.bass_utils import run_bass_kernel_spmd

# ---- problem constants (hardcoded; kernel.py must be self-contained) ----
N = 100000
E = 1600000
B = 64
NODE_DIM, CFG_DIM, HW_DIM, G_DIM = 32, 16, 8, 16
HID, HEADS = 128, 4
HD = HID // HEADS
IN_DIM = NODE_DIM + CFG_DIM + HW_DIM          # 56
NC = 8
NLOC = N // NC                                 # 12500
NB = 98                                        # dst blocks per core
NPAD = NB * 128                                # 12544
RW = 132                                       # table row width: xp|asrc
F = 136                                        # build width: xp|asrc|adst
P = 128
f32 = mybir.dt.float32
bf16 = mybir.dt.bfloat16
i32 = mybir.dt.int32


# --------------------------------------------------------------------------
# host-side prep
# --------------------------------------------------------------------------
def _prep_graph(edge_index):
    """Group non-self-loop edges by destination core and 128-node block."""
    src = np.asarray(edge_index[0]).astype(np.int64)
    dst = np.asarray(edge_index[1]).astype(np.int64)
    core = dst // NLOC
    per_core = []
    tmax = 0
    for p in range(NC):
        sel = core == p
        d_loc = (dst[sel] - p * NLOC).astype(np.int32)
        s_glob = src[sel].astype(np.int64)
        blk = d_loc >> 7
        counts = np.bincount(blk, minlength=NB)
        tmax = max(tmax, int(np.ceil(counts.max() / 128)))
        per_core.append((d_loc, s_glob, blk, counts))
    T = tmax
    epb = T * 128
    outs = []
    for p in range(NC):
        d_loc, s_glob, blk, counts = per_core[p]
        order = np.argsort(blk, kind="stable")
        c_arr = np.full(NB * epb, -1.0, np.float32)
        g_arr = np.zeros(NB * epb, np.int32)
        starts = np.zeros(NB + 1, np.int64)
        np.cumsum(counts, out=starts[1:])
        pos_in_blk = np.arange(len(order)) - starts[blk[order]]
        slot = blk[order] * epb + pos_in_blk
        c_arr[slot] = (d_loc[order] & 127).astype(np.float32)
        gidx = (s_glob // NLOC) * NPAD + (s_glob % NLOC)
        g_arr[slot] = gidx[order].astype(np.int32)
        # device layout: element (partition, b*T + t) = edge slot b*epb + t*128 + p
        c_dev = c_arr.reshape(NB, T, 128).transpose(2, 0, 1).reshape(128, NB * T)
        g_dev = g_arr.reshape(NB, T, 128).transpose(2, 0, 1).reshape(128, NB * T)
        # cLf: cL in edge-slot order for the S^T build ([1, NB*T*128] row)
        outs.append((np.ascontiguousarray(c_dev).astype(ml_dtypes.bfloat16),
                     np.ascontiguousarray(g_dev),
                     np.ascontiguousarray(c_arr.reshape(1, NB * epb))))
    return T, outs


def _prep_inputs(inputs):
    x = np.asarray(inputs["x"], np.float32)
    s2 = np.asarray(inputs["s"], np.float32).reshape(B, CFG_DIM)
    h2 = np.asarray(inputs["h"], np.float32).reshape(B, HW_DIM)
    g2 = np.asarray(inputs["g"], np.float32).reshape(B, G_DIM)
    batch = np.asarray(inputs["batch"], np.int64)

    T, graph = _prep_graph(inputs["edge_index"])

    def wcat(W, as_, ad_):
        W = np.asarray(W, np.float32)
        wa_s = np.stack([W[:, h * HD:(h + 1) * HD] @ np.asarray(as_, np.float32)[h]
                         for h in range(HEADS)], axis=1)
        wa_d = np.stack([W[:, h * HD:(h + 1) * HD] @ np.asarray(ad_, np.float32)[h]
                         for h in range(HEADS)], axis=1)
        return np.concatenate([W, wa_s, wa_d], axis=1).astype(ml_dtypes.bfloat16)

    com = {
        "iota128": np.tile(np.arange(P, dtype=np.float32), (P, 1)).astype(
            ml_dtypes.bfloat16),
        "iotaP": np.arange(P, dtype=np.float32).reshape(P, 1),
        "ones1": np.ones((1, P), np.float32),
        "iota64": np.tile(np.arange(B, dtype=np.float32), (P, 1)),
        "ident": np.eye(P, dtype=np.float32).astype(ml_dtypes.bfloat16),
        "identf": np.eye(P, dtype=np.float32),
        "Win": np.asarray(inputs["Win"], np.float32),
        "binc": np.asarray(inputs["bin_"], np.float32).reshape(P, 1),
        "Wc0": wcat(inputs["W0"], inputs["as0"], inputs["ad0"]),
        "Wc1": wcat(inputs["W1"], inputs["as1"], inputs["ad1"]),
        "b0r": np.tile(np.asarray(inputs["b0"], np.float32), (P, 1)),
        "b1r": np.tile(np.asarray(inputs["b1"], np.float32), (P, 1)),
        "Wh1a": np.asarray(inputs["Wh1"], np.float32)[:HID],
        "Wh1b": np.ascontiguousarray(np.asarray(inputs["Wh1"], np.float32)[HID:]),
        "bh1r": np.tile(np.asarray(inputs["bh1"], np.float32), (B, 1)),
        "Wh2a": np.asarray(inputs["Wh2"], np.float32)[:128],
        "Wh2b": np.ascontiguousarray(np.asarray(inputs["Wh2"], np.float32)[128:]),
        "bh2r": np.tile(np.asarray(inputs["bh2"], np.float32), (B, 1)),
        "Wh3": np.asarray(inputs["Wh3"], np.float32),
        "bh3r": np.tile(np.asarray(inputs["bh3"], np.float32), (B, 1)),
        "sgT": np.ascontiguousarray(np.concatenate([s2, h2, g2], axis=1).T),
    }

    sh = np.concatenate([s2[batch], h2[batch]], axis=1).astype(np.float32)  # [N,24]
    in_maps = []
    for p in range(NC):
        lo, hi = p * NLOC, (p + 1) * NLOC
        xT = np.zeros((NODE_DIM, NPAD), np.float32)
        xT[:, :NLOC] = x[lo:hi].T
        shT = np.zeros((CFG_DIM + HW_DIM, NPAD), np.float32)
        shT[:, :NLOC] = sh[lo:hi].T
        bat = np.full(NPAD, 999.0, np.float32)
        bat[:NLOC] = batch[lo:hi].astype(np.float32)
        c_dev, g_dev, c_f = graph[p]
        m = dict(com)
        m["xT"] = xT
        m["shT"] = shT
        m["bat"] = np.ascontiguousarray(bat.reshape(NB, 128).T)
        m["cL"] = c_dev
        m["gi"] = g_dev
        m["cLf"] = c_f
        in_maps.append(m)
    return T, in_maps


# --------------------------------------------------------------------------
# device program
# --------------------------------------------------------------------------
@functools.lru_cache(maxsize=8)
def _build(T):
    nc = bacc.Bacc("TRN2", target_bir_lowering=False, debug=False)
    ET = NB * T  # gather tiles per core

    def din(name, shape, dt=f32):
        return nc.dram_tensor(name, list(shape), dt, kind="ExternalInput")

    xT_h = din("xT", (NODE_DIM, NPAD))
    shT_h = din("shT", (CFG_DIM + HW_DIM, NPAD))
    bat_h = din("bat", (P, NB))
    cL_h = din("cL", (P, ET), bf16)
    gi_h = din("gi", (P, ET), i32)
    cLf_h = din("cLf", (1, ET * P))
    iota128_h = din("iota128", (P, P), bf16)
    iotaP_h = din("iotaP", (P, 1))
    ones1_h = din("ones1", (1, P))
    iota64_h = din("iota64", (P, B))
    ident_h = din("ident", (P, P), bf16)
    identf_h = din("identf", (P, P))
    Win_h = din("Win", (IN_DIM, HID))
    binc_h = din("binc", (P, 1))
    Wc_h = [din("Wc0", (HID, F), bf16), din("Wc1", (HID, F), bf16)]
    br_h = [din("b0r", (P, HID)), din("b1r", (P, HID))]
    Wh1a_h = din("Wh1a", (HID, 256))
    Wh1b_h = din("Wh1b", (40, 256))
    bh1r_h = din("bh1r", (B, 256))
    Wh2a_h = din("Wh2a", (128, 128))
    Wh2b_h = din("Wh2b", (128, 128))
    bh2r_h = din("bh2r", (B, 128))
    Wh3_h = din("Wh3", (128, 1))
    bh3r_h = din("bh3r", (B, 1))
    sgT_h = din("sgT", (40, B))

    out_h = nc.dram_tensor("out", [B, 1], f32, kind="ExternalOutput")

    xg_loc = [nc.dram_tensor(f"xg{k}_loc", [NPAD, RW], bf16) for k in range(2)]
    xg_full = [nc.dram_tensor(f"xg{k}_full", [NC * NPAD, RW], bf16,
                              addr_space="Shared") for k in range(2)]
    pool_loc = nc.dram_tensor("pool_loc", [B, HID], f32)
    pool_full = nc.dram_tensor("pool_full", [B, HID], f32, addr_space="Shared")

    AG = "AllGather"
    AR = "AllReduce"
    rg = [list(range(NC))]

    with tile.TileContext(nc) as tc, ExitStack() as ctx:
        const = ctx.enter_context(tc.tile_pool(name="const", bufs=1))
        inp = ctx.enter_context(tc.tile_pool(name="inp", bufs=3))
        tblp = ctx.enter_context(tc.tile_pool(name="tblp", bufs=4))
        gxp = ctx.enter_context(tc.tile_pool(name="gxp", bufs=4))
        sp = ctx.enter_context(tc.tile_pool(name="sp", bufs=3))
        stp = ctx.enter_context(tc.tile_pool(name="stp", bufs=3))
        clfp = ctx.enter_context(tc.tile_pool(name="clfp", bufs=3))
        ealp = ctx.enter_context(tc.tile_pool(name="ealp", bufs=3))
        nmp = ctx.enter_context(tc.tile_pool(name="nmp", bufs=3))
        smallp = ctx.enter_context(tc.tile_pool(name="smallp", bufs=4))
        psA = ctx.enter_context(tc.tile_pool(name="psA", bufs=2, space="PSUM"))
        psT = ctx.enter_context(tc.tile_pool(name="psT", bufs=2, space="PSUM"))
        psB = ctx.enter_context(tc.tile_pool(name="psB", bufs=1, space="PSUM"))
        psD = ctx.enter_context(tc.tile_pool(name="psD", bufs=1, space="PSUM"))
        psP = ctx.enter_context(tc.tile_pool(name="psP", bufs=1, space="PSUM"))

        def load_const(h, shape, dt=f32):
            t = const.tile(list(shape), dt, tag=h.name)
            nc.sync.dma_start(t[:], h[:])
            return t

        xhT = const.tile([P, NPAD], bf16, tag="xhT")      # resident activations^T
        localT = const.tile([P, NB, RW], bf16, tag="localT")  # own table rows
        asrcL = const.tile([P, NB * HEADS], f32, tag="asrcL")
        adstL = const.tile([P, NB * HEADS], bf16, tag="adstL")
        adstLf = const.tile([P, NB * HEADS], f32, tag="adstLf")
        easL = const.tile([P, NB * HEADS], bf16, tag="easL")  # self-loop ealpha

        cL = load_const(cL_h, (P, ET), bf16)
        gi = load_const(gi_h, (P, ET), i32)
        bat = load_const(bat_h, (P, NB))
        iota128 = load_const(iota128_h, (P, P), bf16)
        iotaP = load_const(iotaP_h, (P, 1))
        ones1 = load_const(ones1_h, (1, P))
        iota64 = load_const(iota64_h, (P, B))
        ident = load_const(ident_h, (P, P), bf16)
        identf = load_const(identf_h, (P, P))
        WinA = const.tile([NODE_DIM, HID], f32, tag="WinA")
        nc.sync.dma_start(WinA[:], Win_h[0:NODE_DIM, :])
        WinB = const.tile([CFG_DIM + HW_DIM, HID], f32, tag="WinB")
        nc.sync.dma_start(WinB[:], Win_h[NODE_DIM:IN_DIM, :])
        binc = load_const(binc_h, (P, 1))
        Wc = [load_const(Wc_h[k], (HID, F), bf16) for k in range(2)]
        br = [load_const(br_h[k], (P, HID)) for k in range(2)]
        Wh1a = load_const(Wh1a_h, (HID, 256))
        Wh1b = load_const(Wh1b_h, (40, 256))
        bh1r = load_const(bh1r_h, (B, 256))
        Wh2a = load_const(Wh2a_h, (128, 128))
        Wh2b = load_const(Wh2b_h, (128, 128))
        bh2r = load_const(bh2r_h, (B, 128))
        Wh3 = load_const(Wh3_h, (128, 1))
        bh3r = load_const(bh3r_h, (B, 1))
        sgT = load_const(sgT_h, (40, B))

        # ---------------- stage 0: input MLP (feature-major) ----------------
        CH = 512
        for c0 in range(0, NPAD, CH):
            w = min(CH, NPAD - c0)
            xc = inp.tile([NODE_DIM, CH], f32, tag="xc")
            nc.sync.dma_start(xc[:, :w], xT_h[:, c0:c0 + w])
            sc = inp.tile([CFG_DIM + HW_DIM, CH], f32, tag="sc")
            nc.sync.dma_start(sc[:, :w], shT_h[:, c0:c0 + w])
            pin = psT.tile([P, CH], f32, tag="ps1", space="PSUM")
            nc.tensor.matmul(pin[:, :w], lhsT=WinA[:], rhs=xc[:, :w],
                             start=True, stop=False)
            nc.tensor.matmul(pin[:, :w], lhsT=WinB[:], rhs=sc[:, :w],
                             start=False, stop=True)
            nc.scalar.activation(xhT[:, c0:c0 + w], pin[:, :w],
                                 mybir.ActivationFunctionType.Relu, bias=binc[:])

        pool_ps = psP.tile([B, HID], f32, space="PSUM")

        # ---------------- two GAT convs ----------------
        for k in range(2):
            # A) local table rows [xp | asrc] bf16 (adst kept resident only)
            for j in range(NB):
                pt = psT.tile([P, F], f32, tag="ps1", space="PSUM")
                nc.tensor.matmul(pt[:], lhsT=xhT[:, j * P:(j + 1) * P],
                                 rhs=Wc[k][:], start=True, stop=True)
                nc.vector.tensor_copy(localT[:, j, :], pt[:, 0:RW])
                nc.scalar.copy(asrcL[:, j * HEADS:(j + 1) * HEADS],
                               pt[:, HID:HID + HEADS])
                nc.scalar.copy(adstL[:, j * HEADS:(j + 1) * HEADS],
                               pt[:, HID + HEADS:F])
                nc.scalar.copy(adstLf[:, j * HEADS:(j + 1) * HEADS],
                               pt[:, HID + HEADS:F])
                tb = tblp.tile([P, RW], bf16, tag="tb")
                nc.vector.tensor_copy(tb[:], pt[:, 0:RW])
                nc.sync.dma_start(xg_loc[k][j * P:(j + 1) * P, :], tb[:])

            # self-loop ealpha for every local node: exp(lrelu(asrc+adst))
            easf = smallp.tile([P, NB * HEADS], f32, tag="easf")
            nc.vector.tensor_tensor(out=easf[:], in0=asrcL[:], in1=adstLf[:],
                                    op=mybir.AluOpType.add)
            eas2 = smallp.tile([P, NB * HEADS], f32, tag="eas2")
            nc.vector.tensor_scalar(eas2[:], easf[:], 0.2, None,
                                    mybir.AluOpType.mult)
            nc.vector.tensor_tensor(out=easf[:], in0=easf[:], in1=eas2[:],
                                    op=mybir.AluOpType.max)
            nc.scalar.activation(easL[:], easf[:],
                                 mybir.ActivationFunctionType.Exp)

            # B) AllGather the table
            nc.gpsimd.collective_compute(
                AG, mybir.AluOpType.bypass, replica_groups=rg,
                ins=[xg_loc[k][:]], outs=[xg_full[k][:]],
            )

            # C) aggregate per destination block
            for b in range(NB):
                gx = gxp.tile([P, T, RW], bf16, tag="gx")
                for t in range(T):
                    nc.gpsimd.indirect_dma_start(
                        out=gx[:, t, :], out_offset=None, in_=xg_full[k][:],
                        in_offset=bass.IndirectOffsetOnAxis(
                            ap=gi[:, b * T + t:b * T + t + 1], axis=0),
                    )
                # S^T[m, e] = (cL[e] == m): PE K=1 broadcast + DVE is_equal
                clf = clfp.tile([1, T * P], f32, tag="clf")
                nc.sync.dma_start(clf[:], cLf_h[0:1, b * T * P:(b + 1) * T * P])
                S_T = stp.tile([P, T * P], bf16, tag="S_T")
                for c0 in range(0, T * P, 512):
                    w = min(512, T * P - c0)
                    pb = psB.tile([P, 512], f32, tag="pb", space="PSUM")
                    nc.tensor.matmul(pb[:, :w], lhsT=ones1[:],
                                     rhs=clf[:, c0:c0 + w], start=True, stop=True)
                    nc.vector.tensor_scalar(S_T[:, c0:c0 + w], pb[:, :w],
                                            iotaP[:], None,
                                            mybir.AluOpType.is_equal)
                # adst per edge: [e,h] = sum_m S_T[m,e] * adstL[m,h]
                pd = psD.tile([P, T * HEADS], f32, tag="pd", space="PSUM")
                for t in range(T):
                    nc.tensor.matmul(pd[:, t * HEADS:(t + 1) * HEADS],
                                     lhsT=S_T[:, t * P:(t + 1) * P],
                                     rhs=adstL[:, b * HEADS:(b + 1) * HEADS],
                                     start=True, stop=True)
                # attention coefs: asrc[src] + adst[dst]; leakyrelu; exp
                eal = ealp.tile([P, T * HEADS], f32, tag="eal")
                nc.vector.tensor_tensor(
                    out=eal[:].rearrange("p (t h) -> p t h", t=T),
                    in0=gx[:, :, HID:RW],
                    in1=pd[:].rearrange("p (t h) -> p t h", t=T),
                    op=mybir.AluOpType.add)
                t02 = ealp.tile([P, T * HEADS], f32, tag="t02")
                nc.vector.tensor_scalar(t02[:], eal[:], 0.2, None,
                                        mybir.AluOpType.mult)
                nc.vector.tensor_tensor(out=eal[:], in0=eal[:], in1=t02[:],
                                        op=mybir.AluOpType.max)
                # exp -> bf16 into gx cols 128:132
                nc.scalar.activation(gx[:, :, HID:RW], eal[:].rearrange(
                    "p (t h) -> p t h", t=T), mybir.ActivationFunctionType.Exp)
                # selection matrix S[e, m] (bf16)
                S = sp.tile([P, T * P], bf16, tag="S")
                nc.vector.tensor_tensor(
                    out=S[:].rearrange("p (t m) -> p t m", t=T),
                    in0=iota128[:].unsqueeze(1).to_broadcast([P, T, P]),
                    in1=cL[:, b * T:(b + 1) * T].unsqueeze(2).to_broadcast(
                        [P, T, P]),
                    op=mybir.AluOpType.is_equal)
                # scale messages in place
                nc.vector.tensor_tensor(
                    out=gx[:, :, 0:HID].rearrange("p t (h d) -> p t h d", h=HEADS),
                    in0=gx[:, :, 0:HID].rearrange("p t (h d) -> p t h d", h=HEADS),
                    in1=gx[:, :, HID:RW].unsqueeze(3).to_broadcast(
                        [P, T, HEADS, HD]),
                    op=mybir.AluOpType.mult)
                # accumulate messages + denominators
                pa = psA.tile([P, RW], f32, tag="pa", space="PSUM")
                for t in range(T):
                    nc.tensor.matmul(pa[:], lhsT=S[:, t * P:(t + 1) * P],
                                     rhs=gx[:, t, 0:RW],
                                     start=(t == 0), stop=(t == T - 1))
                # epilogue with self-loop terms:
                # num = pa[:,0:128] + eas*xp_own ; den = pa[:,128:132] + eas
                xn = nmp.tile([P, RW], f32, tag="xn")
                nc.vector.tensor_tensor(
                    out=xn[:, 0:HID].rearrange("p (h d) -> p h d", h=HEADS),
                    in0=localT[:, b, 0:HID].rearrange("p (h d) -> p h d", h=HEADS),
                    in1=easL[:, b * HEADS:(b + 1) * HEADS].unsqueeze(2)
                        .to_broadcast([P, HEADS, HD]),
                    op=mybir.AluOpType.mult)
                nc.vector.tensor_tensor(out=xn[:, 0:HID], in0=xn[:, 0:HID],
                                        in1=pa[:, 0:HID],
                                        op=mybir.AluOpType.add)
                den = smallp.tile([P, HEADS], f32, tag="den")
                nc.vector.tensor_tensor(out=den[:], in0=pa[:, HID:RW],
                                        in1=easL[:, b * HEADS:(b + 1) * HEADS],
                                        op=mybir.AluOpType.add)
                denr = smallp.tile([P, HEADS], f32, tag="denr")
                nc.vector.reciprocal(denr[:], den[:])
                for h in range(HEADS):
                    nc.vector.tensor_scalar(xn[:, h * HD:(h + 1) * HD],
                                            xn[:, h * HD:(h + 1) * HD],
                                            denr[:, h:h + 1], None,
                                            mybir.AluOpType.mult)
                nc.vector.tensor_tensor(out=xn[:, 0:HID], in0=xn[:, 0:HID],
                                        in1=br[k][:], op=mybir.AluOpType.add)
                xnf = nmp.tile([P, HID], f32, tag="xnf")
                nc.scalar.activation(xnf[:], xn[:, 0:HID],
                                     mybir.ActivationFunctionType.Relu)
                if k == 0:
                    ptr = psT.tile([P, P], f32, tag="ps1", space="PSUM")
                    nc.tensor.transpose(out=ptr[:], in_=xnf[:], identity=identf[:])
                    nc.scalar.copy(xhT[:, b * P:(b + 1) * P], ptr[:])
                else:
                    Spool = smallp.tile([P, B], f32, tag="Spool")
                    nc.vector.tensor_scalar(Spool[:], iota64[:], bat[:, b:b + 1],
                                            None, mybir.AluOpType.is_equal)
                    nc.tensor.matmul(pool_ps[:], lhsT=Spool[:], rhs=xnf[:],
                                     start=(b == 0), stop=(b == NB - 1))

        # ---------------- pooling all-reduce + head MLP ----------------
        psb = smallp.tile([B, HID], f32, tag="psb")
        nc.scalar.copy(psb[:], pool_ps[:])
        nc.sync.dma_start(pool_loc[:], psb[:])
        nc.gpsimd.collective_compute(
            AR, mybir.AluOpType.add, replica_groups=rg,
            ins=[pool_loc[:]], outs=[pool_full[:]],
        )
        pooled = smallp.tile([B, HID], f32, tag="pooled")
        nc.sync.dma_start(pooled[:], pool_full[:])

        ptp = psT.tile([P, B], f32, tag="ps1", space="PSUM")
        nc.tensor.transpose(out=ptp[:], in_=pooled[:], identity=identf[0:B, 0:B])
        pT = smallp.tile([P, B], f32, tag="pT")
        nc.scalar.copy(pT[:], ptp[:])

        z1p = psT.tile([B, 256], f32, tag="ps1", space="PSUM")
        nc.tensor.matmul(z1p[:], lhsT=pT[:], rhs=Wh1a[:], start=True, stop=False)
        nc.tensor.matmul(z1p[:], lhsT=sgT[:], rhs=Wh1b[:], start=False, stop=True)
        z1 = smallp.tile([B, 256], f32, tag="z1")
        nc.vector.tensor_tensor(out=z1[:], in0=z1p[:], in1=bh1r[:],
                                op=mybir.AluOpType.add)
        nc.scalar.activation(z1[:], z1[:], mybir.ActivationFunctionType.Relu)

        z1Ta = psT.tile([P, B], f32, tag="ps1", space="PSUM")
        nc.tensor.transpose(out=z1Ta[:], in_=z1[:, 0:128], identity=identf[0:B, 0:B])
        z1Tb = psT.tile([P, B], f32, tag="ps1", space="PSUM")
        nc.tensor.transpose(out=z1Tb[:], in_=z1[:, 128:256], identity=identf[0:B, 0:B])
        z1Tas = smallp.tile([P, B], f32, tag="z1Tas")
        nc.scalar.copy(z1Tas[:], z1Ta[:])
        z1Tbs = smallp.tile([P, B], f32, tag="z1Tbs")
        nc.scalar.copy(z1Tbs[:], z1Tb[:])

        z2p = psT.tile([B, 128], f32, tag="ps1", space="PSUM")
        nc.tensor.matmul(z2p[:], lhsT=z1Tas[:], rhs=Wh2a[:], start=True, stop=False)
        nc.tensor.matmul(z2p[:], lhsT=z1Tbs[:], rhs=Wh2b[:], start=False, stop=True)
        z2 = smallp.tile([B, 128], f32, tag="z2")
        nc.vector.tensor_tensor(out=z2[:], in0=z2p[:], in1=bh2r[:],
                                op=mybir.AluOpType.add)
        nc.scalar.activation(z2[:], z2[:], mybir.ActivationFunctionType.Relu)

        z2T = psT.tile([P, B], f32, tag="ps1", space="PSUM")
        nc.tensor.transpose(out=z2T[:], in_=z2[:], identity=identf[0:B, 0:B])
        z2Ts = smallp.tile([P, B], f32, tag="z2Ts")
        nc.scalar.copy(z2Ts[:], z2T[:])

        z3p = psT.tile([B, 1], f32, tag="ps1", space="PSUM")
        nc.tensor.matmul(z3p[:], lhsT=z2Ts[:], rhs=Wh3[:], start=True, stop=True)
        z3 = smallp.tile([B, 1], f32, tag="z3")
        nc.vector.tensor_tensor(out=z3[:], in0=z3p[:], in1=bh3r[:],
                                op=mybir.AluOpType.add)
        nc.sync.dma_start(out_h[:], z3[:])

    nc.compile()
    return nc


# --------------------------------------------------------------------------
# entry point
# --------------------------------------------------------------------------
def kernel(**inputs) -> np.ndarray:
    T, in_maps = _prep_inputs(inputs)
    nc = _build(T)
    res = run_bass_kernel_spmd(nc, in_maps, core_ids=list(range(NC)))
    return np.asarray(res.results[0]["out"], np.float32).reshape(B)
